# revision 1
# baseline (speedup 1.0000x reference)
"""GaussianMixture log-likelihood kernel for 8 TRN2 NeuronCores.

Math (per point x, cluster k):
  S_k = L_k L_k^T  (L = cov_inv_sqrt),  coef_k = pr_k * |det L_k|
  d_ik = -0.5 (x-c_k)^T S_k (x-c_k) = -0.5 || L_k^T x - b_k ||^2,  b_k = L_k^T c_k
  ll_i = log sum_k coef_k exp(d_ik)  - threshold

Device strategy (data-parallel over N, 8192 points/core):
  - Host builds X~ = [X | 1] and G_k = [[L_k],[-b_k^T]] in R^{65 x 64} so that
    Z'_i,(k,c) = (X~ @ G)_ic has  d_ik = -0.5 * sum_c Z'^2.
  - Per 128-point block: PE transpose of X~ block -> stationary [65,128],
    4 matmuls (float32r, N=512) -> Z [128, 2048] in PSUM,
    ACT square-evac (scale sqrt(0.5), bf16) -> DVE segmented reduce -> U=d.
  - Epilogue: E=exp(U+EXPB) (ACT), E*=coef (DVE), s=sum_k (DVE), Ln (ACT),
    -EXPB-threshold (DVE), PE transpose, DMA out.
"""

import sys

sys.path.insert(0, "/opt/trn_rl_repo")

import numpy as np

from concourse import bacc, bass, mybir
from concourse.tile import TileContext
from concourse.bass_utils import run_bass_kernel_spmd

N, D, K = 65536, 64, 32
NCORES = 8
NLOC = N // NCORES            # 8192 points per core
BLK = 128                     # points per block (partition dim)
NBLK = NLOC // BLK            # 64 blocks per core
GRP = 4                       # blocks per transpose/DMA group
NGRP = NBLK // GRP            # 16 groups
DA = D + 1                    # augmented contraction dim (65)
KD = K * D                    # 2048 Z columns
EXPB = 84.0                   # exp bias: exp(d + EXPB), d <= 0 always

F32 = mybir.dt.float32
F32R = mybir.dt.float32r
BF16 = mybir.dt.bfloat16


def _build_nc(threshold_f: float):
    nc = bacc.Bacc()

    CW = KD + BLK
    CF = K + 1 + BLK
    xa_d = nc.declare_dram_parameter("xa", [NLOC, DA], BF16, isOutput=False)
    cs_d = nc.declare_dram_parameter("consts", [BLK, CW], BF16, isOutput=False)
    cfe_d = nc.declare_dram_parameter("cfe", [BLK, CF], F32, isOutput=False)
    out_d = nc.declare_dram_parameter("out", [NBLK, BLK], F32, isOutput=True)

    with TileContext(nc) as tc:
        with (
            tc.tile_pool(name="const", bufs=1) as cpool,
            tc.tile_pool(name="x4", bufs=NGRP) as xpool,
            tc.tile_pool(name="xt4", bufs=NGRP) as xtpool,
            tc.tile_pool(name="z2", bufs=4) as z2pool,
            tc.tile_pool(name="big", bufs=1) as bigpool,
            tc.tile_pool(name="fin", bufs=1) as finpool,
            tc.tile_pool(name="pst", bufs=2, space="PSUM") as pstpool,
            tc.tile_pool(name="psz", bufs=2, space="PSUM") as pszpool,
            tc.tile_pool(name="pscr", bufs=1, space="PSUM") as scrpool,
        ):
            consts = cpool.tile([BLK, CW], BF16)
            nc.sync.dma_start(out=consts[:, :], in_=cs_d[:, :])
            cfe = cpool.tile([BLK, CF], F32)
            nc.sync.dma_start(out=cfe[:, :], in_=cfe_d[:, :])
            wt = consts[:DA, 0:KD]
            ident = consts[:, KD : KD + BLK]
            cf = cfe[:, 0:K]
            ebias = cfe[:, K : K + 1]
            identf = cfe[:, K + 1 : K + 1 + BLK]

            U = bigpool.tile([BLK, NBLK * K], F32)  # d values, [128, b(64), k(32)]

            for g in range(NGRP):
                # load 4 blocks: [128 p, 4 j, 65 d]
                x4 = xpool.tile([BLK, GRP, DA], BF16)
                nc.sync.dma_start(
                    out=x4[:, :, :],
                    in_=xa_d[g * GRP * BLK : (g + 1) * GRP * BLK, :].rearrange(
                        "(j p) d -> p j d", j=GRP, p=BLK
                    ),
                )
                # dummy PE op consuming x4 so real transposes carry <=1 wait
                # (f32r Matmult LDWEIGHTS allows a single sync-wait slot)
                scr = scrpool.tile([BLK, BLK], BF16)
                nc.tensor.transpose(scr[:DA, :DA], x4[:, 0, :], x4[:, 0, :].rearrange("p d -> p d"))
                # transpose the 4 blocks into one psum bank [65, 512]
                pst = pstpool.tile([BLK, GRP * BLK], BF16)
                for j in range(GRP):
                    nc.tensor.transpose(
                        pst[:DA, j * BLK : (j + 1) * BLK], x4[:, j, :], ident
                    )
                xt4 = xtpool.tile([DA, GRP * BLK], BF16)
                nc.scalar.copy(out=xt4[:, :], in_=pst[:DA, :])

                for j in range(GRP):
                    b = g * GRP + j
                    lhsT = xt4[:, j * BLK : (j + 1) * BLK]
                    for h in range(2):
                        z = pszpool.tile([BLK, 1024], F32)
                        for q in range(2):
                            nc.tensor.matmul(
                                z[:, q * 512 : (q + 1) * 512],
                                lhsT,
                                wt[:, h * 1024 + q * 512 : h * 1024 + (q + 1) * 512],
                                start=True,
                                stop=True,
                            )
                        # square-evac: 0.5 * z^2 in bf16
                        z2 = z2pool.tile([BLK, 1024], BF16)
                        nc.scalar.activation(
                            out=z2[:, :],
                            in_=z[:, :],
                            func=mybir.ActivationFunctionType.Square,
                            scale=float(np.sqrt(0.5)),
                        )
                        # segmented reduce over c=64 -> U[:, b, 16h:16h+16] = d
                        nc.vector.tensor_reduce(
                            out=U[:, b * K + h * 16 : b * K + h * 16 + 16],
                            in_=z2.rearrange("p (k c) -> p k c", c=D),
                            axis=mybir.AxisListType.X,
                            op=mybir.AluOpType.add,
                            negate=True,
                        )

            # ---- epilogue (stable logsumexp) ----
            m = finpool.tile([BLK, NBLK], F32)
            nc.vector.tensor_reduce(
                out=m[:, :],
                in_=U.rearrange("p (b k) -> p b k", k=K),
                axis=mybir.AxisListType.X,
                op=mybir.AluOpType.max,
            )
            V = bigpool.tile([BLK, NBLK * K], F32)
            nc.vector.tensor_tensor(
                out=V.rearrange("p (b k) -> p b k", k=K),
                in0=U.rearrange("p (b k) -> p b k", k=K),
                in1=m[:, :].unsqueeze(2).broadcast_to([BLK, NBLK, K]),
                op=mybir.AluOpType.subtract,
            )
            E = bigpool.tile([BLK, NBLK * K], F32)
            nc.scalar.activation(
                out=E[:, :], in_=V[:, :],
                func=mybir.ActivationFunctionType.Exp,
            )
            EC = bigpool.tile([BLK, NBLK * K], F32)
            nc.vector.tensor_tensor(
                out=EC.rearrange("p (b k) -> p b k", k=K),
                in0=E.rearrange("p (b k) -> p b k", k=K),
                in1=cf.unsqueeze(1).broadcast_to([BLK, NBLK, K]),
                op=mybir.AluOpType.mult,
            )
            s = finpool.tile([BLK, NBLK], F32)
            nc.vector.tensor_reduce(
                out=s[:, :],
                in_=EC.rearrange("p (b k) -> p b k", k=K),
                axis=mybir.AxisListType.X,
                op=mybir.AluOpType.add,
            )
            lls = finpool.tile([BLK, NBLK], F32)
            nc.scalar.activation(
                out=lls[:, :], in_=s[:, :],
                func=mybir.ActivationFunctionType.Ln,
            )
            llf = finpool.tile([BLK, NBLK], F32)
            nc.vector.scalar_tensor_tensor(
                out=llf[:, :], in0=lls[:, :], scalar=-threshold_f,
                in1=m[:, :], op0=mybir.AluOpType.add, op1=mybir.AluOpType.add,
            )
            # transpose [128 p, 64 b] -> [64 b, 128 p] for a contiguous DMA out
            pso = scrpool.tile([BLK, BLK], F32, tag="scr")
            nc.tensor.transpose(pso[:NBLK, :BLK], llf[:, :], identf)
            llT = finpool.tile([NBLK, BLK], F32)
            nc.scalar.copy(out=llT[:, :], in_=pso[:NBLK, :BLK])
            nc.sync.dma_start(out=out_d[:, :], in_=llT[:, :])

    nc.compile()
    return nc


def _host_prep(X, center, cov_inv_sqrt, weight, threshold):
    L = cov_inv_sqrt.astype(np.float64)
    w = np.abs(weight.astype(np.float64))
    pr = w / w.sum()
    sign, logdetL = np.linalg.slogdet(L)          # det(S)=det(L)^2 -> sqrt=|det L|
    coef = pr * np.exp(logdetL)                   # [K]
    b = np.einsum("kde,kd->ke", L, center.astype(np.float64))  # b_k = L_k^T c_k

    G = np.zeros((DA, KD), np.float64)
    for k in range(K):
        G[:D, k * D : (k + 1) * D] = L[k]
        G[D, k * D : (k + 1) * D] = -b[k]

    Xa = np.concatenate([X, np.ones((N, 1), X.dtype)], axis=1)  # [N, 65]

    import ml_dtypes
    BFD = ml_dtypes.bfloat16
    CW = KD + BLK
    CF = K + 1 + BLK
    consts = np.zeros((BLK, CW), BFD)
    consts[:DA, 0:KD] = G.astype(BFD)
    consts[:, KD : KD + BLK] = np.eye(BLK, dtype=BFD)
    cfe = np.zeros((BLK, CF), np.float32)
    cfe[:, 0:K] = np.tile(coef[None, :].astype(np.float32), (BLK, 1))
    cfe[:, K] = EXPB
    cfe[:, K + 1 :] = np.eye(BLK, dtype=np.float32)
    thr = float(np.asarray(threshold, dtype=np.float64))
    return Xa.astype(BFD), consts, cfe, thr


_CACHE = {}


def kernel(X, center, cov_inv_sqrt, weight, threshold):
    Xa, consts, cfe, thr = _host_prep(X, center, cov_inv_sqrt, weight, threshold)

    key = ("nc", thr)
    if key not in _CACHE:
        _CACHE[key] = _build_nc(thr)
    nc = _CACHE[key]

    in_maps = []
    for i in range(NCORES):
        shard = np.ascontiguousarray(Xa[i * NLOC : (i + 1) * NLOC])
        in_maps.append({"xa": shard, "consts": consts, "cfe": cfe})

    res = run_bass_kernel_spmd(nc, in_maps, core_ids=list(range(NCORES)))
    outs = res.results
    ll = np.concatenate(
        [np.asarray(outs[i]["out"], dtype=np.float32).reshape(NLOC) for i in range(NCORES)]
    )
    return ll



# revision 8
# speedup vs baseline: 1.2004x; 1.2004x over previous
"""GaussianMixture log-likelihood kernel for 8 TRN2 NeuronCores.

Math (per point x, cluster k):
  S_k = L_k L_k^T  (L = cov_inv_sqrt),  coef_k = pr_k * |det L_k|
  d_ik = -0.5 (x-c_k)^T S_k (x-c_k) = -0.5 || L_k^T x - b_k ||^2,  b_k = L_k^T c_k
  ll_i = log sum_k coef_k exp(d_ik)  - threshold

Device strategy (data-parallel over N, 8192 points/core):
  - Host builds Xa^T = [X | 1]^T in [65, 8192] (pre-transposed, so no PE
    transposes on device) and G_k = [[L_k],[-b_k^T]] in R^{65 x 64}.
  - Per 128-point block: 4 matmuls (lhsT = Xa^T block [65,128] stationary,
    rhs = G chunks [65,512]) -> Z [128, 2048] f32 in PSUM (4 banks,
    double-buffered), then ONE ACT Square evac (scale sqrt(0.5), fp16):
    s2 = 0.5 Z^2.
  - Per 4-block group: DVE fold-tree over c (6 stages, fp16 2x mode,
    final stage f32) -> U[p, b, k] = 0.5 ||Z||^2 = -d.
  - Epilogue: E = exp(-U + 84) (ACT free affine), E *= coef (DVE),
    s = sum_k (DVE segmented reduce), Ln (ACT), -84 - threshold (ACT add),
    PE transpose, DMA out.
"""

import sys

sys.path.insert(0, "/opt/trn_rl_repo")

import numpy as np

from concourse import bacc, bass, mybir
from concourse.tile import TileContext
from concourse.bass_utils import run_bass_kernel_spmd

N, D, K = 65536, 64, 32
NCORES = 8
NLOC = N // NCORES            # 8192 points per core
BLK = 128                     # points per block (partition dim)
NBLK = NLOC // BLK            # 64 blocks per core
GRP = 4                       # blocks per fold group
NGRP = NBLK // GRP            # 16 groups
DA = D + 1                    # augmented contraction dim (65)
KD = K * D                    # 2048 Z columns per point
# exp bias: exp(d + EXPB + ln coef). d <= 0 always, ln coef_max ~ -8.
# Upper bound: scalar-engine Ln input must stay within 2^64, so
# EXPB + max(d) + max(ln coef) + ln K < 44  ->  EXPB = 50 is safe.
# Lower bound: sum underflows only if max_k d_k < -(87 + EXPB - 8) ~ -129.
EXPB = 50.0

F32 = mybir.dt.float32
BF16 = mybir.dt.bfloat16
FP16 = mybir.dt.float16
SQ = mybir.ActivationFunctionType.Square
EXP = mybir.ActivationFunctionType.Exp
LN = mybir.ActivationFunctionType.Ln


def _build_nc(threshold_f: float):
    nc = bacc.Bacc()

    xat_d = nc.declare_dram_parameter("xat", [DA, NLOC], BF16, isOutput=False)
    g_d = nc.declare_dram_parameter("g", [DA, KD], BF16, isOutput=False)
    cf_d = nc.declare_dram_parameter("cf", [BLK, K + 2], F32, isOutput=False)
    idf_d = nc.declare_dram_parameter("idf", [BLK, BLK], F32, isOutput=False)
    out_d = nc.declare_dram_parameter("out", [NBLK, BLK], F32, isOutput=True)

    XCH = NLOC // 4  # xa^T DMA chunk: 2048 points (16 blocks)

    with TileContext(nc) as tc:
        with (
            tc.tile_pool(name="const", bufs=1) as cpool,
            tc.tile_pool(name="xat", bufs=4) as xpool,
            tc.tile_pool(name="s2", bufs=2) as s2pool,
            tc.tile_pool(name="fold", bufs=2) as fpool,
            tc.tile_pool(name="big", bufs=1) as bigpool,
            tc.tile_pool(name="fin", bufs=1) as finpool,
        ):
            g = cpool.tile([DA, KD], BF16)
            nc.sync.dma_start(out=g[:, :], in_=g_d[:, :])
            cfe = cpool.tile([BLK, K + 2], F32)
            nc.sync.dma_start(out=cfe[:, :], in_=cf_d[:, :])
            cf = cfe[:, 0:K]
            ebias = cfe[:, K : K + 1]          # EXPB
            fbias = cfe[:, K + 1 : K + 2]      # -(EXPB + threshold)
            idf = cpool.tile([BLK, BLK], F32)
            nc.sync.dma_start(out=idf[:, :], in_=idf_d[:, :])

            xat = []
            for q in range(4):
                xt = xpool.tile([DA, XCH], BF16)
                nc.sync.dma_start(
                    out=xt[:, :], in_=xat_d[:, q * XCH : (q + 1) * XCH]
                )
                xat.append(xt)

            U = bigpool.tile([BLK, NBLK * K], F32)  # 0.5||Z||^2, [128, b(64), k(32)]

            with tc.tile_pool(name="psz", bufs=2, space="PSUM") as zpool:
                for gi in range(NGRP):
                    s2 = s2pool.tile([BLK, GRP, KD], FP16)
                    for j in range(GRP):
                        b = gi * GRP + j
                        lhsT = xat[b // 16][:, (b % 16) * BLK : (b % 16) * BLK + BLK]
                        z = zpool.tile([BLK, KD], F32)
                        for q in range(4):
                            nc.tensor.matmul(
                                z[:, q * 512 : (q + 1) * 512],
                                lhsT,
                                g[:, q * 512 : (q + 1) * 512],
                                start=True,
                                stop=True,
                            )
                        # square-evac: 0.5 * z^2 in fp16, one ACT instr
                        nc.scalar.activation(
                            out=s2[:, j, :], in_=z[:, :], func=SQ,
                            scale=float(np.sqrt(0.5)),
                        )
                    # fold tree over c: 64 -> 1, fp16 2x mode (final f32)
                    JK = GRP * K  # 128 (j,k) groups
                    v0 = s2.rearrange("p j (k c) -> p (j k) c", c=D)
                    t1 = fpool.tile([BLK, JK, 32], FP16)
                    nc.vector.tensor_tensor(
                        out=t1, in0=v0[:, :, 0:32], in1=v0[:, :, 32:64],
                        op=mybir.AluOpType.add,
                    )
                    t2 = fpool.tile([BLK, JK, 16], FP16)
                    nc.vector.tensor_tensor(
                        out=t2, in0=t1[:, :, 0:16], in1=t1[:, :, 16:32],
                        op=mybir.AluOpType.add,
                    )
                    t3 = fpool.tile([BLK, JK, 8], FP16)
                    nc.vector.tensor_tensor(
                        out=t3, in0=t2[:, :, 0:8], in1=t2[:, :, 8:16],
                        op=mybir.AluOpType.add,
                    )
                    t4 = fpool.tile([BLK, JK, 4], FP16)
                    nc.vector.tensor_tensor(
                        out=t4, in0=t3[:, :, 0:4], in1=t3[:, :, 4:8],
                        op=mybir.AluOpType.add,
                    )
                    t5 = fpool.tile([BLK, JK, 2], FP16)
                    nc.vector.tensor_tensor(
                        out=t5, in0=t4[:, :, 0:2], in1=t4[:, :, 2:4],
                        op=mybir.AluOpType.add,
                    )
                    nc.vector.tensor_tensor(
                        out=U[:, gi * GRP * K : (gi + 1) * GRP * K].rearrange(
                            "p (jk c) -> p jk c", c=1
                        ),
                        in0=t5[:, :, 0:1], in1=t5[:, :, 1:2],
                        op=mybir.AluOpType.add,
                    )

            # ---- epilogue: ll = ln(sum_k coef_k exp(-U + EXPB)) - EXPB - thr
            E = bigpool.tile([BLK, NBLK * K], F32)
            nc.scalar.activation(
                out=E[:, :], in_=U[:, :], func=EXP, scale=-1.0, bias=ebias,
            )
            EC = bigpool.tile([BLK, NBLK * K], F32)
            nc.vector.tensor_tensor(
                out=EC.rearrange("p (b k) -> p b k", k=K),
                in0=E.rearrange("p (b k) -> p b k", k=K),
                in1=cf.unsqueeze(1).broadcast_to([BLK, NBLK, K]),
                op=mybir.AluOpType.mult,
            )
            s = finpool.tile([BLK, NBLK], F32)
            nc.vector.tensor_reduce(
                out=s[:, :],
                in_=EC.rearrange("p (b k) -> p b k", k=K),
                axis=mybir.AxisListType.X,
                op=mybir.AluOpType.add,
            )
            lls = finpool.tile([BLK, NBLK], F32)
            nc.scalar.activation(out=lls[:, :], in_=s[:, :], func=LN)
            llf = finpool.tile([BLK, NBLK], F32)
            nc.scalar.add(llf[:, :], lls[:, :], fbias)

            with tc.tile_pool(name="pso", bufs=1, space="PSUM") as opool:
                pso = opool.tile([BLK, BLK], F32)
                nc.tensor.transpose(pso[:NBLK, :BLK], llf[:, :], idf)
                llT = finpool.tile([NBLK, BLK], F32)
                nc.scalar.copy(out=llT[:, :], in_=pso[:NBLK, :BLK])
                nc.sync.dma_start(out=out_d[:, :], in_=llT[:, :])

    nc.compile()
    return nc


def _host_prep(X, center, cov_inv_sqrt, weight, threshold):
    L = cov_inv_sqrt.astype(np.float64)
    w = np.abs(weight.astype(np.float64))
    pr = w / w.sum()
    sign, logdetL = np.linalg.slogdet(L)          # det(S)=det(L)^2 -> sqrt=|det L|
    coef = pr * np.exp(logdetL)                   # [K]
    b = np.einsum("kde,kd->ke", L, center.astype(np.float64))  # b_k = L_k^T c_k

    G = np.zeros((DA, KD), np.float64)
    for k in range(K):
        G[:D, k * D : (k + 1) * D] = L[k]
        G[D, k * D : (k + 1) * D] = -b[k]

    import ml_dtypes
    BFD = ml_dtypes.bfloat16
    XaT = np.empty((DA, N), np.float32)
    XaT[:D] = X.T
    XaT[D] = 1.0
    thr = float(np.asarray(threshold, dtype=np.float64))
    cfm = np.zeros((BLK, K + 2), np.float32)
    cfm[:, 0:K] = coef[None, :].astype(np.float32)
    cfm[:, K] = EXPB
    cfm[:, K + 1] = -(EXPB + thr)
    idf = np.eye(BLK, dtype=np.float32)
    return XaT.astype(BFD), G.astype(BFD), np.ascontiguousarray(cfm), idf, thr


_CACHE = {}


def kernel(X, center, cov_inv_sqrt, weight, threshold):
    XaT, G, cfm, idf, thr = _host_prep(X, center, cov_inv_sqrt, weight, threshold)

    key = ("nc", thr)
    if key not in _CACHE:
        _CACHE[key] = _build_nc(thr)
    nc = _CACHE[key]

    in_maps = []
    for i in range(NCORES):
        shard = np.ascontiguousarray(XaT[:, i * NLOC : (i + 1) * NLOC])
        in_maps.append({"xat": shard, "g": G, "cf": cfm, "idf": idf})

    res = run_bass_kernel_spmd(nc, in_maps, core_ids=list(range(NCORES)))
    outs = res.results
    ll = np.concatenate(
        [np.asarray(outs[i]["out"], dtype=np.float32).reshape(NLOC) for i in range(NCORES)]
    )
    return ll
